# revision 5
# baseline (speedup 1.0000x reference)
"""Trainium2 Bass kernel for nn_DecoderTF (masked spectrogram decode + overlap-add).

Computation (per batch m, channel c):
    masked[n, k] = inputs[m, n, k] * est_mask[m, c, n, k]          n in [0,512), k in [0,6000)
    frames[k, l] = sum_n masked[n, k] * W[n, l]                    l in [0,16)
    out[m, c, t] = overlap_and_add(frames, hop=8)                  t in [0,48008)

With L=16 and hop=8, overlap-add is a two-term sum; per output row k (out2d[k, j]
= out[8k + j]):
    out2d[k, j] = frames[k, j] + frames[k-1, 8 + j]

Pipeline per 512-column k-slice, per channel:
  1. one matmul group: stationary W [128n, 16] x moving mk [128n, 512k] -> PSUM
     P[16, 512] (all 16 W columns in one pass -- each mk column streams through
     the PE exactly once, half the streaming of an 8-wide-stationary split).
  2. ACT copies the B-half P[8:16] to SBUF (same partitions), a small SBUF->SBUF
     DMA remaps it to partitions 0:8 shifted one column right (DVE lanes cannot
     cross partitions, DMA can).
  3. DVE adds A (straight from PSUM) + shifted B -> the finished out2d rows in
     [8, 512] layout, zero-padded to 32 partitions.
  4. DVE stream-transpose (32x32 blocks) puts k on partitions; the output DMA
     stores 32-byte rows. No PE transposes anywhere: the PE only runs 4 big
     matmuls per slice-channel.

DMA tiles are exactly 512 k-columns wide (2 KB rows, no halo column), so no
4-byte runt packets on the HBM stream.

Sharding: data-parallel over M -- core m handles inputs[m] / est_mask[m] (no
cross-core communication, W replicated).  Per-core HBM traffic ~37 MB.
"""

import sys

for _p in ("/opt/trn_rl_repo",):
    if _p not in sys.path:
        sys.path.insert(0, _p)

import numpy as np

import concourse.bass as bass
import concourse.mybir as mybir
from concourse import bacc
from concourse.tile import TileContext
from concourse.bass_utils import run_bass_kernel_spmd

N, L, HOP = 512, 16, 8
K = 6000
C = 2
M = 8
T_OUT = (K - 1) * HOP + L  # 48008

F32 = mybir.dt.float32
# float32r streams fp32 bits through the PE single-pass path: at >=256 moving
# columns it runs 1 cycle/column, same as bf16, with fp32-grade accuracy.
MM_DT = mybir.dt.float32r

SW = 512  # k-slice width: one PSUM bank, 2 KB DMA rows
SLICES = [(i * SW, SW) for i in range(11)] + [(11 * SW, K - 11 * SW)]  # last 368


def _build_nc():
    nc = bacc.Bacc()
    x = nc.declare_dram_parameter("x", [N, K], F32, isOutput=False)
    mk = nc.declare_dram_parameter("mask", [C, N, K], F32, isOutput=False)
    w = nc.declare_dram_parameter("w", [N, L], MM_DT, isOutput=False)
    out = nc.declare_dram_parameter("out", [C, T_OUT], F32, isOutput=True)

    with TileContext(nc) as tc:
        with (
            tc.tile_pool(name="wp", bufs=1) as wp,
            tc.tile_pool(name="xp", bufs=4) as xp,
            tc.tile_pool(name="mp", bufs=4) as mp,
            tc.tile_pool(name="mkp", bufs=6) as mkp,
            tc.tile_pool(name="bsp", bufs=3) as bsp,
            tc.tile_pool(name="fbp", bufs=4) as fbp,
            tc.tile_pool(name="f2p", bufs=3) as f2p,
            tc.tile_pool(name="trp", bufs=3) as trp,
            tc.tile_pool(name="pfp", bufs=4, space="PSUM") as pfp,
        ):
            # W in lhsT layout: w_t[p, 16n + l] = W[128n + p, l]; the slice
            # w_t[:, 16n : 16n+16] is the full 16-wide stationary for chunk n.
            w_t = wp.tile([128, 4 * L], MM_DT)
            nc.sync.dma_start(
                out=w_t[:, :].rearrange("p (n l) -> p n l", n=4),
                in_=w.rearrange("(n p) l -> p n l", p=128),
            )

            prev_frB = {0: None, 1: None}
            prev_w = 0
            f2_cnt = 0
            for si, (o0, wks) in enumerate(SLICES):
                last = si == len(SLICES) - 1

                x_t = xp.tile([128, 4 * SW], F32, tag="x")
                nc.sync.dma_start(
                    out=x_t[:, :].rearrange("p (n k) -> p n k", n=4)[:, :, 0:wks],
                    in_=x.rearrange("(n p) k -> p n k", p=128)[:, :, o0 : o0 + wks],
                )
                m_t = mp.tile([128, 2 * 4 * SW], F32, tag="m")
                m_v = m_t[:, :].rearrange("p (c n k) -> p c n k", c=2, n=4)
                for c in range(C):
                    nc.sync.dma_start(
                        out=m_v[:, c, :, 0:wks],
                        in_=mk.rearrange("c (n p) k -> p c n k", p=128)[
                            :, c, :, o0 : o0 + wks
                        ],
                    )

                for c in range(C):
                    mk_t = mkp.tile([128, 4 * SW], MM_DT, tag="mk")
                    nc.vector.tensor_mul(
                        mk_t[:, :].rearrange("p (n k) -> p n k", n=4)[:, :, 0:wks],
                        x_t[:, :].rearrange("p (n k) -> p n k", n=4)[:, :, 0:wks],
                        m_v[:, c, :, 0:wks],
                    )

                    pf = pfp.tile([16, SW], F32, tag="pf")
                    for n in range(4):
                        nc.tensor.matmul(
                            pf[0:16, 0:wks],
                            w_t[:, 16 * n : 16 * n + 16],
                            mk_t[:, SW * n : SW * n + wks],
                            start=(n == 0),
                            stop=(n == 3),
                        )

                    # B-half to SBUF (same partitions), then partition-remap
                    # 8:16 -> 0:8 with a one-column right shift via SB->SB DMA
                    # on the (otherwise idle) SWDGE ring.
                    # engine partition ranges must start 32-aligned, so copy
                    # both halves (same column count -> same ACT cycles)
                    bst = bsp.tile([16, SW], F32, tag="bs")
                    nc.scalar.copy(bst[0:16, 0:wks], pf[0:16, 0:wks])
                    frB = fbp.tile([8, SW + 8], F32, tag="fb")
                    if si == 0:
                        # frames[-1] is zero: zero halo for the first slice
                        nc.vector.memset(frB[0:8, 0:1], 0.0)
                    else:
                        nc.scalar.copy(
                            frB[0:8, 0:1],
                            prev_frB[c][0:8, prev_w : prev_w + 1],
                        )
                    nc.scalar.dma_start(
                        out=frB[0:8, 1 : 1 + wks], in_=bst[8:16, 0:wks]
                    )

                    # out2d rows of this slice in [8, w] layout:
                    #   fr2[j, t] = P_A[j, t] + P_B[j, t-1]
                    wout = wks + 1 if last else wks  # extra B-only final row
                    wpad = (wout + 31) // 32 * 32
                    fr2 = f2p.tile([32, SW], F32, tag="f2")
                    if f2_cnt < 3:
                        nc.vector.memset(fr2[:, :], 0.0)
                        f2_cnt += 1
                    nc.vector.tensor_add(
                        fr2[0:8, 0:wks], pf[0:8, 0:wks], frB[0:8, 0:wks]
                    )
                    if last:
                        nc.vector.tensor_copy(
                            fr2[0:8, wks : wks + 1], frB[0:8, wks : wks + 1]
                        )

                    tr = trp.tile([32, SW], F32, tag="tr")
                    nc.vector.transpose(tr[0:32, 0:wpad], fr2[0:32, 0:wpad])

                    # store: out[c, 8*(o0 + 32b + p) + j] = tr[p, 32b + j]
                    nfull = wout // 32
                    rem = wout - 32 * nfull
                    t0 = 8 * o0
                    if nfull:
                        nc.scalar.dma_start(
                            out=out[c, t0 : t0 + 256 * nfull].rearrange(
                                "(b p j) -> p b j", p=32, j=8
                            ),
                            in_=tr[:, 0 : 32 * nfull].rearrange(
                                "p (b j) -> p b j", j=32
                            )[:, :, 0:8],
                        )
                    if rem:
                        nc.scalar.dma_start(
                            out=out[
                                c, t0 + 256 * nfull : t0 + 256 * nfull + 8 * rem
                            ].rearrange("(p j) -> p j", j=8),
                            in_=tr[0:rem, 32 * nfull : 32 * nfull + 8],
                        )
                    prev_frB[c] = frB
                prev_w = wks
    nc.finalize()
    return nc


_NC_CACHE = None


def _get_nc():
    global _NC_CACHE
    if _NC_CACHE is None:
        _NC_CACHE = _build_nc()
    return _NC_CACHE


def run(inputs, est_mask, W, trace=False):
    """Returns (out [M, C, T_OUT] float32, exec_time_ns or None)."""
    inputs = np.ascontiguousarray(np.asarray(inputs, dtype=np.float32))
    est_mask = np.ascontiguousarray(np.asarray(est_mask, dtype=np.float32))
    W = np.ascontiguousarray(np.asarray(W, dtype=np.float32))
    assert inputs.shape == (M, N, K)
    assert est_mask.shape == (M, C, N, K)
    assert W.shape == (N, L)

    nc = _get_nc()
    in_maps = [
        {"x": inputs[m], "mask": est_mask[m], "w": W} for m in range(M)
    ]
    res = run_bass_kernel_spmd(nc, in_maps, list(range(M)), trace=trace)
    out = np.stack([res.results[m]["out"] for m in range(M)], axis=0)
    return out.astype(np.float32, copy=False), res.exec_time_ns


def kernel(inputs, est_mask, W):
    out, _ = run(inputs, est_mask, W)
    return out


# revision 6
# speedup vs baseline: 1.0171x; 1.0171x over previous
"""Trainium2 Bass kernel for nn_DecoderTF (masked spectrogram decode + overlap-add).

Computation (per batch m, channel c):
    masked[n, k] = inputs[m, n, k] * est_mask[m, c, n, k]          n in [0,512), k in [0,6000)
    frames[k, l] = sum_n masked[n, k] * W[n, l]                    l in [0,16)
    out[m, c, t] = overlap_and_add(frames, hop=8)                  t in [0,48008)

With L=16 and hop=8, overlap-add is a two-term sum; per output row k (out2d[k, j]
= out[8k + j]):
    out2d[k, j] = frames[k, j] + frames[k-1, 8 + j]

Pipeline per 512-column k-slice, per channel:
  1. one matmul group: stationary W [128n, 16] x moving mk [128n, 512k] -> PSUM
     P[16, 512] (all 16 W columns in one pass -- each mk column streams through
     the PE exactly once, half the streaming of an 8-wide-stationary split).
  2. ACT copies P to SBUF; a small SBUF->SBUF DMA remaps the B-half from
     partitions 8:16 to 0:8 shifted one column right (DVE lanes cannot cross
     partitions, DMA can).
  3. DVE adds A (straight from PSUM) + shifted B -> the finished out2d rows in
     [8, 512] layout, zero-padded to 32 partitions.
  4. DVE stream-transpose (32x32 blocks) puts k on partitions; the output DMA
     stores 32-byte rows. No PE transposes anywhere: the PE only runs 4 big
     matmuls per slice-channel.

Engine queues execute in emission order, so late pipeline stages are emitted
with a slice of delay (preadd/transpose one slice late, output DMA two slices
late): by the time an engine reaches a delayed instruction its inputs are long
since ready and nothing blocks the younger work queued behind it.

DMA tiles are exactly 512 k-columns wide (2 KB rows, no halo column), so no
4-byte runt packets on the HBM stream.

Sharding: data-parallel over M -- core m handles inputs[m] / est_mask[m] (no
cross-core communication, W replicated).  Per-core HBM traffic ~37 MB.
"""

import sys

for _p in ("/opt/trn_rl_repo",):
    if _p not in sys.path:
        sys.path.insert(0, _p)

import numpy as np

import concourse.bass as bass
import concourse.mybir as mybir
from concourse import bacc
from concourse.tile import TileContext
from concourse.bass_utils import run_bass_kernel_spmd

N, L, HOP = 512, 16, 8
K = 6000
C = 2
M = 8
T_OUT = (K - 1) * HOP + L  # 48008

F32 = mybir.dt.float32
# float32r streams fp32 bits through the PE single-pass path: at >=256 moving
# columns it runs 1 cycle/column, same as bf16, with fp32-grade accuracy.
MM_DT = mybir.dt.float32r

SW = 512  # k-slice width: one PSUM bank, 2 KB DMA rows
SLICES = [(i * SW, SW) for i in range(11)] + [(11 * SW, K - 11 * SW)]  # last 368


def _build_nc():
    nc = bacc.Bacc()
    x = nc.declare_dram_parameter("x", [N, K], F32, isOutput=False)
    mk = nc.declare_dram_parameter("mask", [C, N, K], F32, isOutput=False)
    w = nc.declare_dram_parameter("w", [N, L], MM_DT, isOutput=False)
    out = nc.declare_dram_parameter("out", [C, T_OUT], F32, isOutput=True)

    with TileContext(nc) as tc:
        with (
            tc.tile_pool(name="wp", bufs=1) as wp,
            tc.tile_pool(name="xp", bufs=5) as xp,
            tc.tile_pool(name="mp", bufs=4) as mp,
            tc.tile_pool(name="mkp", bufs=7) as mkp,
            tc.tile_pool(name="bsp", bufs=4) as bsp,
            tc.tile_pool(name="fbp", bufs=4) as fbp,
            tc.tile_pool(name="f2p", bufs=3) as f2p,
            tc.tile_pool(name="trp", bufs=6) as trp,
            tc.tile_pool(name="pfp", bufs=6, space="PSUM") as pfp,
        ):
            # W in lhsT layout: w_t[p, 16n + l] = W[128n + p, l]; the slice
            # w_t[:, 16n : 16n+16] is the full 16-wide stationary for chunk n.
            w_t = wp.tile([128, 4 * L], MM_DT)
            nc.sync.dma_start(
                out=w_t[:, :].rearrange("p (n l) -> p n l", n=4),
                in_=w.rearrange("(n p) l -> p n l", p=128),
            )

            state = {}  # per (si, c): dict of tiles
            f2_cnt = [0]

            def emit_input(si):
                o0, wks = SLICES[si]
                x_t = xp.tile([128, 4 * SW], F32, tag="x")
                nc.sync.dma_start(
                    out=x_t[:, :].rearrange("p (n k) -> p n k", n=4)[:, :, 0:wks],
                    in_=x.rearrange("(n p) k -> p n k", p=128)[
                        :, :, o0 : o0 + wks
                    ],
                )
                m_t = mp.tile([128, 2 * 4 * SW], F32, tag="m")
                m_v = m_t[:, :].rearrange("p (c n k) -> p c n k", c=2, n=4)
                for c in range(C):
                    nc.sync.dma_start(
                        out=m_v[:, c, :, 0:wks],
                        in_=mk.rearrange("c (n p) k -> p c n k", p=128)[
                            :, c, :, o0 : o0 + wks
                        ],
                    )
                state[si] = {"x": x_t, "mv": m_v}

            def emit_mult_mm(si, c):
                o0, wks = SLICES[si]
                st = state[si]
                mk_t = mkp.tile([128, 4 * SW], MM_DT, tag="mk")
                nc.vector.tensor_mul(
                    mk_t[:, :].rearrange("p (n k) -> p n k", n=4)[:, :, 0:wks],
                    st["x"][:, :].rearrange("p (n k) -> p n k", n=4)[
                        :, :, 0:wks
                    ],
                    st["mv"][:, c, :, 0:wks],
                )
                pf = pfp.tile([16, SW], F32, tag="pf")
                for n in range(4):
                    nc.tensor.matmul(
                        pf[0:16, 0:wks],
                        w_t[:, 16 * n : 16 * n + 16],
                        mk_t[:, SW * n : SW * n + wks],
                        start=(n == 0),
                        stop=(n == 3),
                    )
                st[c, "pf"] = pf

            def emit_bshift(si, c):
                o0, wks = SLICES[si]
                st = state[si]
                pf = st[c, "pf"]
                # engine partition ranges must start 32-aligned, so copy both
                # halves (same column count -> same ACT cycles)
                bst = bsp.tile([16, SW], F32, tag="bs")
                nc.scalar.copy(bst[0:16, 0:wks], pf[0:16, 0:wks])
                frB = fbp.tile([8, SW + 8], F32, tag="fb")
                if si == 0:
                    # frames[-1] is zero: zero halo for the first slice
                    nc.vector.memset(frB[0:8, 0:1], 0.0)
                else:
                    pw = SLICES[si - 1][1]
                    nc.scalar.copy(
                        frB[0:8, 0:1], state[si - 1][c, "frB"][0:8, pw : pw + 1]
                    )
                nc.scalar.dma_start(
                    out=frB[0:8, 1 : 1 + wks], in_=bst[8:16, 0:wks]
                )
                st[c, "frB"] = frB

            def emit_add_transpose(si, c):
                o0, wks = SLICES[si]
                st = state[si]
                pf, frB = st[c, "pf"], st[c, "frB"]
                last = si == len(SLICES) - 1
                wout = wks + 1 if last else wks  # extra B-only final row
                wpad = (wout + 31) // 32 * 32
                fr2 = f2p.tile([32, SW], F32, tag="f2")
                if f2_cnt[0] < 3:
                    nc.vector.memset(fr2[:, :], 0.0)
                    f2_cnt[0] += 1
                nc.vector.tensor_add(
                    fr2[0:8, 0:wks], pf[0:8, 0:wks], frB[0:8, 0:wks]
                )
                if last:
                    nc.vector.tensor_copy(
                        fr2[0:8, wks : wks + 1], frB[0:8, wks : wks + 1]
                    )
                tr = trp.tile([32, SW], F32, tag="tr")
                nc.vector.transpose(tr[0:32, 0:wpad], fr2[0:32, 0:wpad])
                st[c, "tr"] = tr
                st[c, "wout"] = wout

            def emit_out(si, c):
                o0, _ = SLICES[si]
                st = state[si]
                tr, wout = st[c, "tr"], st[c, "wout"]
                # store: out[c, 8*(o0 + 32b + p) + j] = tr[p, 32b + j]
                nfull = wout // 32
                rem = wout - 32 * nfull
                t0 = 8 * o0
                if nfull:
                    nc.scalar.dma_start(
                        out=out[c, t0 : t0 + 256 * nfull].rearrange(
                            "(b p j) -> p b j", p=32, j=8
                        ),
                        in_=tr[:, 0 : 32 * nfull].rearrange(
                            "p (b j) -> p b j", j=32
                        )[:, :, 0:8],
                    )
                if rem:
                    nc.scalar.dma_start(
                        out=out[
                            c, t0 + 256 * nfull : t0 + 256 * nfull + 8 * rem
                        ].rearrange("(p j) -> p j", j=8),
                        in_=tr[0:rem, 32 * nfull : 32 * nfull + 8],
                    )

            # Software-pipelined emission: per-engine queues run in order, so
            # stage S of slice si is emitted alongside earlier stages of later
            # slices -- a delayed instruction's inputs are ready before the
            # engine reaches it, and it never blocks younger work behind it.
            NS = len(SLICES)
            for si in range(NS + 2):
                if si < NS:
                    emit_input(si)
                    for c in range(C):
                        emit_mult_mm(si, c)
                        emit_bshift(si, c)
                if 1 <= si <= NS:
                    for c in range(C):
                        emit_add_transpose(si - 1, c)
                if si >= 2:
                    for c in range(C):
                        emit_out(si - 2, c)
                if si >= 3:
                    del state[si - 3]
    nc.finalize()
    return nc


_NC_CACHE = None


def _get_nc():
    global _NC_CACHE
    if _NC_CACHE is None:
        _NC_CACHE = _build_nc()
    return _NC_CACHE


def run(inputs, est_mask, W, trace=False):
    """Returns (out [M, C, T_OUT] float32, exec_time_ns or None)."""
    inputs = np.ascontiguousarray(np.asarray(inputs, dtype=np.float32))
    est_mask = np.ascontiguousarray(np.asarray(est_mask, dtype=np.float32))
    W = np.ascontiguousarray(np.asarray(W, dtype=np.float32))
    assert inputs.shape == (M, N, K)
    assert est_mask.shape == (M, C, N, K)
    assert W.shape == (N, L)

    nc = _get_nc()
    in_maps = [
        {"x": inputs[m], "mask": est_mask[m], "w": W} for m in range(M)
    ]
    res = run_bass_kernel_spmd(nc, in_maps, list(range(M)), trace=trace)
    out = np.stack([res.results[m]["out"] for m in range(M)], axis=0)
    return out.astype(np.float32, copy=False), res.exec_time_ns


def kernel(inputs, est_mask, W):
    out, _ = run(inputs, est_mask, W)
    return out
